# revision 20
# baseline (speedup 1.0000x reference)
"""AttentionBlock (GroupNorm + degenerate head-axis attention + proj + residual)
on 8 Trainium2 NeuronCores, data-parallel over batch (2 batches per core).

Reference math (B=16, C=256, H=W=64, NH=4, dh=64, N=HW=4096, G=8 groups):
  xn   = GroupNorm(8, C)(x) * norm_w + norm_b       (xn = D x + s per channel)
  qkv  = qkv_w @ xn + qkv_b                         (1x1 convs == channel GEMMs)
  q,k,v: [NH, dh, N]; attn[p,i,j] = softmax_j( (1/8) sum_n q[i,p,n] k[j,p,n] )
  out[(p,i), n] = sum_j attn[p,i,j] v[(p,j), n]
  y    = proj_w @ out + proj_b + x

v2 kernel strategy (per core, per batch) — Gram-based:
  - xT is produced by DMA-transpose (xbar) straight from DRAM: xt[p,t,c] =
    x[c, t*128+p]; the PE accumulates the raw-x channel Gram G = X X^T in
    PSUM from the 32 transposed chunks (f16 operands, f32 accumulate)
  - GroupNorm never touches the big data: the diag scale D folds into the
    tiny q/k weight GEMMs, A = (Wq D) G (Wk D)^T + exact rank-1 bias terms
    (f32r full-rate matmuls), softmax over the pm-block-diagonal as before
  - attention+value+proj collapse to M2T[c,o] (tiny GEMMs); then BOTH the
    GroupNorm scale and the residual fold into it:  M2tot = D M2T + I, so
      y = M2tot^T @ x + pbeff      (one big GEMM over raw f16 x, no x_s,
    no identity-residual matmul, all biases in the per-channel pbeff)
  - per-channel stats for D come from bn_stats on the normal-layout x copy,
    which the y-GEMM needs anyway as its streaming operand
"""
import numpy as np

import concourse.bacc as bacc
import concourse.mybir as mybir
import concourse.tile as tile
from concourse.bass_utils import run_bass_kernel_spmd

F32 = mybir.dt.float32
F32R = mybir.dt.float32r
F16 = mybir.dt.float16

NCORES = 8
B, C, H, W = 16, 256, 64, 64
N = H * W                    # 4096
NB = B // NCORES             # batches per core = 2
NH, DH, G = 4, 64, 8
EPS = 1e-5
P = 128
NCH = C // P                 # channel chunks = 2
NT128 = N // 128             # 32
MASK_NEG = -1.0e30

PRECISION = "f16"            # output/io dtype tag (kept for test.py printout)

# p-major channel permutation: pm index j = p*4 + h  <->  orig channel h*64 + p
_PM = np.arange(C)
ORIG_OF_PM = (_PM % NH) * DH + _PM // NH   # orig channel for p-major index

TQ = 4                       # transpose DMA chunks per batch
YQ = 4                       # y store chunks per (batch, oc)
XR_ENG = "pool"              # "pool" (SWDGE) | "hwdge"
Y_ENG = "pool"               # "pool" | "hwdge"
FAKE_XT = False              # timing-only: skip transposes, memset xt once
NOSTATS = False              # timing-only: memset stat2 instead of bn_stats
GRAM_T = NT128               # timing-only: number of gram accumulation steps
Y_M = NCH                    # timing-only: contraction chunks in y GEMM
NOYD = False                 # timing-only: skip y stores
TMODE = "pe"                 # "pe" | "dram" | "sbuf" transpose method
NOCHAIN = False              # timing-only: DMAs + gram only, dummy y store
TSPLIT = True                # issue DMA transposes on both HWDGE rings
NOBC = False                 # timing-only: skip stages B/C, dummy m2tot/pbeff


def _build(replicate=1, loop=1, prec=None):
    """loop>1 wraps the computation in a hardware For_i repeating it
    (identical result every iteration) — used only for wall-clock timing."""
    nc = bacc.Bacc()
    x_d = nc.declare_dram_parameter("x", [NB, C, N], F16, isOutput=False)
    wqk_d = nc.declare_dram_parameter("wqk", [C, 512], F32R, isOutput=False)
    wv_d = nc.declare_dram_parameter("wv", [C, C], F32R, isOutput=False)
    wvr_d = nc.declare_dram_parameter("wvr", [C, C], F16, isOutput=False)
    pt_d = nc.declare_dram_parameter("pt", [C, C], F16, isOutput=False)
    bqk_d = nc.declare_dram_parameter("bqk", [1, 512], F32, isOutput=False)
    bv_d = nc.declare_dram_parameter("bv", [P, NCH], F32, isOutput=False)
    pb_d = nc.declare_dram_parameter("pb", [P, NCH], F32, isOutput=False)
    nw_d = nc.declare_dram_parameter("nw", [P, NCH], F32, isOutput=False)
    nb_d = nc.declare_dram_parameter("nb", [P, NCH], F32, isOutput=False)
    ind_d = nc.declare_dram_parameter("ind", [P, NCH, G], F32, isOutput=False)
    bc_d = nc.declare_dram_parameter("bc", [G, NCH, P], F32, isOutput=False)
    mask_d = nc.declare_dram_parameter("mask", [P, P], F32, isOutput=False)
    rmask_d = nc.declare_dram_parameter("rmask", [P, NCH, C], F32, isOutput=False)
    ident_d = nc.declare_dram_parameter("ident", [P, P], F16, isOutput=False)
    y_d = nc.declare_dram_parameter("y", [NB, C, N], F16, isOutput=True)

    AOT = mybir.AluOpType
    AFT = mybir.ActivationFunctionType

    def f32view(ap):
        return ap.bitcast(F32) if ap.dtype == F32R else ap

    with tile.TileContext(nc) as tc:
        with (
            tc.tile_pool(name="wpool", bufs=1) as wpool,
            tc.tile_pool(name="xt", bufs=2) as xt_pool,       # transposed x
            tc.tile_pool(name="xr", bufs=2) as xr_pool,       # normal x
            tc.tile_pool(name="per_b", bufs=2) as pb_pool,    # per-batch smalls
            tc.tile_pool(name="sm", bufs=3) as sm_pool,       # softmax temps
            tc.tile_pool(name="ypool", bufs=4) as y_pool,
            tc.tile_pool(name="psg", bufs=2, space="PSUM") as ps_gram,   # G / A
            tc.tile_pool(name="psx", bufs=2, space="PSUM") as ps_xp,     # PE transpose
            tc.tile_pool(name="psy", bufs=2, space="PSUM") as ps_y,
            tc.tile_pool(name="pss0", bufs=1, space="PSUM") as ps_small0,
            tc.tile_pool(name="pss1", bufs=1, space="PSUM") as ps_small1,
        ):
            # ---- load constants ----
            wqk_t = wpool.tile([P, NCH, 512], F32R)
            nc.sync.dma_start(wqk_t[:], wqk_d.rearrange("(m p) o -> p m o", p=P))
            wv_t = wpool.tile([P, NCH, C], F32R)
            nc.sync.dma_start(wv_t[:], wv_d.rearrange("(m p) o -> p m o", p=P))
            wvr_t = wpool.tile([P, NCH, C], F16)
            nc.sync.dma_start(wvr_t[:], wvr_d.rearrange("(m p) o -> p m o", p=P))
            pt_t = wpool.tile([P, NCH, C], F16)
            nc.sync.dma_start(pt_t[:], pt_d.rearrange("(m p) o -> p m o", p=P))
            bqk_t = wpool.tile([1, 512], F32)
            nc.sync.dma_start(bqk_t[:], bqk_d[:])
            bv_t = wpool.tile([P, NCH], F32)
            nc.sync.dma_start(bv_t[:], bv_d[:])
            pb_t = wpool.tile([P, NCH], F32)
            nc.sync.dma_start(pb_t[:], pb_d[:])
            nw_t = wpool.tile([P, NCH], F32)
            nc.sync.dma_start(nw_t[:], nw_d[:])
            nb_t = wpool.tile([P, NCH], F32)
            nc.sync.dma_start(nb_t[:], nb_d[:])
            ind_t = wpool.tile([P, NCH, G], F32)
            nc.sync.dma_start(ind_t[:], ind_d[:])
            bc_t = wpool.tile([G, NCH, P], F32)
            nc.sync.dma_start(bc_t[:], bc_d[:])
            mask_t = wpool.tile([P, P], F32)
            nc.sync.dma_start(mask_t[:], mask_d[:])
            rmask_t = wpool.tile([P, NCH, C], F32)
            nc.sync.dma_start(rmask_t[:], rmask_d[:])
            ident_t = wpool.tile([P, P], F16)
            nc.sync.dma_start(ident_t[:], ident_d[:])
            eps_t = wpool.tile([G, 1], F32)
            nc.vector.memset(eps_t[:], EPS)

            import contextlib
            loop_ctx = tc.For_i(0, loop, 1) if loop > 1 else contextlib.nullcontext()
            with loop_ctx:
              for _rep in range(replicate):
                st = [dict() for _ in range(NB)]

                # ------------- input DMAs, both batches, interleaved -------------
                for b in range(NB):
                    s = st[b]
                    s["xt"] = xt_pool.tile([P, NT128, C], F16, tag="xt", name=f"xt_{b}")
                    s["xr"] = xr_pool.tile([P, NCH, N], F16, tag="xr", name=f"xr_{b}")
                xr_eng = {"pool": nc.gpsimd, "hwdge": nc.scalar}[XR_ENG]
                NTQ = NT128 // TQ
                NQ = N // TQ
                for b in range(NB):
                    s = st[b]
                    if FAKE_XT:
                        nc.vector.memset(s["xt"][:, 0:1, :], 0.001)
                    if TMODE == "pe":
                        engs = [nc.gpsimd, nc.sync, nc.scalar]
                        for i, (m, h) in enumerate([(0, 0), (0, 1), (1, 0), (1, 1)]):
                            engs[(i + 2 * b) % 3].dma_start(
                                s["xr"][:, m, h * 2048:(h + 1) * 2048],
                                x_d[b, m * P:(m + 1) * P, h * 2048:(h + 1) * 2048])
                    elif TMODE == "sbuf":
                        for m in range(NCH):
                            for h in range(2):
                                xr_eng.dma_start(
                                    s["xr"][:, m, h * 2048:(h + 1) * 2048],
                                    x_d[b, m * P:(m + 1) * P, h * 2048:(h + 1) * 2048])
                                if not FAKE_XT:
                                    nc.sync.dma_start(
                                        s["xt"][:, h * 16:(h + 1) * 16, m * P:(m + 1) * P],
                                        s["xr"][:, m, h * 2048:(h + 1) * 2048],
                                        transpose=True)
                    else:
                        for q in range(TQ // 2):
                            if not FAKE_XT:
                                teng = nc.scalar if (TSPLIT and q % 2 == 1) else nc.sync
                                teng.dma_start(
                                    s["xt"][:, q * NTQ:(q + 1) * NTQ, :],
                                    x_d[b, :, q * NQ:(q + 1) * NQ], transpose=True)
                        xr_eng.dma_start(s["xr"][:, 0, :], x_d[b, 0:P, :])
                        for q in range(TQ // 2, TQ):
                            if not FAKE_XT:
                                teng = nc.scalar if (TSPLIT and q % 2 == 1) else nc.sync
                                teng.dma_start(
                                    s["xt"][:, q * NTQ:(q + 1) * NTQ, :],
                                    x_d[b, :, q * NQ:(q + 1) * NQ], transpose=True)
                        xr_eng.dma_start(s["xr"][:, 1, :], x_d[b, P:C, :])

                ps_smalls = [ps_small0, ps_small1]

                # ------------- per-batch pipeline -------------
                def stageA(b):
                    """Gram accumulation (PE) + channel stats (DVE)."""
                    s = st[b]
                    xt, x_r = s["xt"], s["xr"]
                    # one accumulation chain per PSUM bank (full-bank tiles):
                    # a start=True matmul clears has_written for the whole 2KB
                    # zero region, so chains must never share a bank.
                    g_ps = [ps_gram.tile([P, 512], F32, tag="ga", name=f"g_{b}_{m}")
                            for m in range(NCH)]
                    if TMODE == "pe" and not FAKE_XT:
                        # two t-chunks per PSUM bank (4 single-MM chains), one
                        # [P, 512] f16 copy per pair
                        for tp in range(NT128 // 2):
                            xp = ps_xp.tile([P, 512], F32, tag="xp",
                                            name=f"xp_{b}_{tp}")
                            for half in range(2):
                                t = 2 * tp + half
                                for m in range(NCH):
                                    nc.tensor.matmul(
                                        xp[:, half * 256 + m * P:
                                           half * 256 + (m + 1) * P],
                                        x_r[:, m, t * P:(t + 1) * P],
                                        ident_t[:], start=True, stop=True)
                            if tp % 2 == 0:
                                nc.vector.tensor_copy(xt[:, 2 * tp:2 * tp + 2, :],
                                                      xp[:])
                            else:
                                nc.scalar.copy(xt[:, 2 * tp:2 * tp + 2, :], xp[:])
                    for t in range(GRAM_T):
                        tt = 0 if FAKE_XT else t % NT128
                        for m in range(NCH):
                            nc.tensor.matmul(g_ps[m][:, 0:256],
                                             xt[:, tt, m * P:(m + 1) * P],
                                             xt[:, tt, :],
                                             start=(t == 0), stop=(t == GRAM_T - 1))
                    s["g_ps"] = g_ps
                    # per-channel stats: stat2 = (mean, E[x^2])
                    stat2 = pb_pool.tile([P, NCH, 2], F32, tag="stat2")
                    if NOSTATS:
                        nc.vector.memset(stat2[:], 1.0)
                    else:
                        for m in range(NCH):
                            stats = pb_pool.tile([P, 8, 6], F32, tag="stats")
                            for j in range(8):
                                nc.vector.bn_stats(stats[:, j, :],
                                                   x_r[:, m, j * 512:(j + 1) * 512])
                            mv = pb_pool.tile([P, 2], F32, tag="mv")
                            nc.vector.bn_aggr(mv[:], stats[:])
                            nc.vector.tensor_copy(stat2[:, m, 0:1], mv[:, 0:1])
                            sq = pb_pool.tile([P, 1], F32, tag="sq")
                            nc.vector.tensor_mul(sq[:], mv[:, 0:1], mv[:, 0:1])
                            nc.vector.tensor_add(stat2[:, m, 1:2], mv[:, 1:2], sq[:])
                    s["stat2"] = stat2

                def stageB(b):
                    """Group stats -> scale/shift; wqkD; G->SBUF; T and A (+rank-1)."""
                    s = st[b]
                    stat2 = s["stat2"]
                    shiftw = pb_pool.tile([P, NCH, 2], F32R, tag="shiftw")
                    sxs2 = pb_pool.tile([P, NCH, 2], F32R, tag="sxs2")
                    wqkD = pb_pool.tile([P, NCH, 512], F32R, tag="wqkD")
                    scale_keep = []
                    # group stats: one chain covers all 8 groups (each group's
                    # channels live in exactly one chunk; ind zeros elsewhere)
                    sg_ps = ps_smalls[b].tile([G, 2], F32, tag="small")
                    for m in range(NCH):
                        nc.tensor.matmul(sg_ps[:], ind_t[:, m, :], stat2[:, m, :],
                                         start=(m == 0), stop=(m == NCH - 1))
                    gs = pb_pool.tile([G, 2], F32, tag="gs")
                    nc.vector.tensor_copy(gs[:], sg_ps[:])
                    gsq = pb_pool.tile([G, 1], F32, tag="gsq")
                    nc.vector.tensor_mul(gsq[:], gs[:, 0:1], gs[:, 0:1])
                    gvar = pb_pool.tile([G, 1], F32, tag="gvar")
                    nc.vector.tensor_tensor(gvar[:], gs[:, 1:2], gsq[:], AOT.subtract)
                    grt = pb_pool.tile([G, 1], F32, tag="grt")
                    nc.scalar.activation(grt[:], gvar[:], AFT.Ln, bias=eps_t[:])
                    grtm = pb_pool.tile([G, 1], F32, tag="grtm")
                    nc.vector.tensor_scalar_mul(grtm[:], grt[:], -0.5)
                    gstats = pb_pool.tile([G, 2], F32, tag="gstats")
                    nc.vector.tensor_copy(gstats[:, 0:1], gs[:, 0:1])
                    nc.scalar.activation(gstats[:, 1:2], grtm[:], AFT.Exp)
                    for m in range(NCH):
                        # per-channel scale/shift for chunk m
                        bc_ps = ps_smalls[b].tile([P, 2], F32, tag="small")
                        nc.tensor.matmul(bc_ps[:], bc_t[:, m, :], gstats[:],
                                         start=True, stop=True)
                        scale_m = pb_pool.tile([P, 1], F32, tag="scale_m")
                        nc.vector.tensor_mul(scale_m[:], bc_ps[:, 1:2], nw_t[:, m:m + 1])
                        tmp_m = pb_pool.tile([P, 1], F32, tag="tmp_m")
                        nc.vector.tensor_mul(tmp_m[:], bc_ps[:, 0:1], scale_m[:])
                        # shift = norm_b - mean*scale (duplicated to width 2)
                        nc.vector.tensor_tensor(shiftw[:, m, 0:1],
                                                nb_t[:, m:m + 1], tmp_m[:], AOT.subtract)
                        nc.vector.tensor_tensor(shiftw[:, m, 1:2],
                                                nb_t[:, m:m + 1], tmp_m[:], AOT.subtract)
                        # sxs = scale * sum_n x = scale * N * mean
                        sxs_m = pb_pool.tile([P, 1], F32, tag="sxs_m")
                        nc.vector.tensor_mul(sxs_m[:], stat2[:, m, 0:1], scale_m[:])
                        nc.vector.tensor_scalar_mul(sxs2[:, m, 0:1], sxs_m[:], float(N))
                        nc.vector.tensor_scalar_mul(sxs2[:, m, 1:2], sxs_m[:], float(N))
                        # D-scaled q/k weights for this contraction chunk
                        nc.vector.tensor_scalar_mul(wqkD[:, m, :],
                                                    f32view(wqk_t[:, m, :]), scale_m[:])
                        scale_keep.append(scale_m)
                    s["shiftw"], s["sxs2"] = shiftw, sxs2
                    s["wqkD"], s["scale"] = wqkD, scale_keep

                    # rank-1 bias vectors (fp32): beff = W.T @ shift + bqk  [1, 512]
                    bq_ps = ps_smalls[b].tile([2, 512], F32, tag="small")
                    for m in range(NCH):
                        nc.tensor.matmul(bq_ps[:], shiftw[:, m, :], wqk_t[:, m, :],
                                         start=(m == 0), stop=(m == NCH - 1))
                    bq_sb = pb_pool.tile([1, 512], F32R, tag="bq_sb")
                    nc.vector.tensor_tensor(bq_sb[:], bq_ps[0:1, :], bqk_t[:], AOT.add)
                    # wsx = W.T @ (scale * sx) -> [1, 512]
                    wsx_ps = ps_smalls[b].tile([2, 512], F32, tag="small")
                    for m in range(NCH):
                        nc.tensor.matmul(wsx_ps[:], sxs2[:, m, :], wqk_t[:, m, :],
                                         start=(m == 0), stop=(m == NCH - 1))
                    wsx_sb = pb_pool.tile([1, 512], F32R, tag="wsx_sb")
                    nc.vector.tensor_copy(wsx_sb[:], wsx_ps[0:1, :])
                    # wcomb = Wk_eff sx + N * beff_k
                    nbk = pb_pool.tile([1, C], F32, tag="nbk")
                    nc.vector.tensor_scalar_mul(nbk[:], f32view(bq_sb[0:1, C:2 * C]),
                                                float(N))
                    wcomb = pb_pool.tile([1, C], F32R, tag="wcomb")
                    nc.vector.tensor_tensor(wcomb[:], f32view(wsx_sb[0:1, C:2 * C]),
                                            nbk[:], AOT.add)
                    s["bq_sb"], s["wsx_sb"], s["wcomb"] = bq_sb, wsx_sb, wcomb

                    # G -> SBUF (fp32)
                    g_sb = pb_pool.tile([P, NCH, 256], F32R, tag="g_sb")
                    for m in range(NCH):
                        nc.vector.tensor_copy(g_sb[:, m, :], s["g_ps"][m][:, 0:256])
                    # T[c2, i] = sum_c1 G[c1, c2] WqD[c1, i]   (q-half only)
                    t_ps = [ps_smalls[b].tile([P, 512], F32, tag="small", name=f"t_{b}_{o}")
                            for o in range(NCH)]
                    for o in range(NCH):
                        for m1 in range(NCH):
                            nc.tensor.matmul(t_ps[o][:, 0:256],
                                             g_sb[:, m1, o * P:(o + 1) * P],
                                             wqkD[:, m1, 0:256],
                                             start=(m1 == 0), stop=(m1 == NCH - 1))
                    t_sb = pb_pool.tile([P, NCH, 256], F32R, tag="t_sb")
                    for o in range(NCH):
                        nc.scalar.copy(t_sb[:, o, :], t_ps[o][:, 0:256])
                    # A[i, j] = sum_c2 T[c2, i] WkD[c2, j]  + rank-1 bias terms
                    a_ps = [ps_smalls[b].tile([P, 512], F32, tag="small", name=f"a_{b}_{m}")
                            for m in range(NCH)]
                    for m in range(NCH):
                        for o in range(NCH):
                            nc.tensor.matmul(a_ps[m][:, 0:256],
                                             t_sb[:, o, m * P:(m + 1) * P],
                                             wqkD[:, o, 256:512],
                                             start=(o == 0), stop=False)
                        nc.tensor.matmul(a_ps[m][:, 0:256],
                                         bq_sb[0:1, m * P:(m + 1) * P],
                                         wcomb[:], start=False, stop=False)
                        nc.tensor.matmul(a_ps[m][:, 0:256],
                                         wsx_sb[0:1, m * P:(m + 1) * P],
                                         bq_sb[0:1, C:2 * C], start=False, stop=True)
                    s["a_ps"] = a_ps

                def stageC(b):
                    """Softmax, QT, M2T (+fold scale & residual), bveff/pbeff."""
                    s = st[b]
                    a_ps = s["a_ps"]
                    qt_t = pb_pool.tile([P, NCH, C], F16, tag="qt")
                    for m in range(NCH):
                        grel = a_ps[m][:, m * P:(m + 1) * P]
                        s_t = sm_pool.tile([P, P], F32, tag="s")
                        nc.vector.tensor_tensor(s_t[:], grel, mask_t[:], AOT.add)
                        e_t = sm_pool.tile([P, P], F32, tag="e")
                        esum = sm_pool.tile([P, 1], F32, tag="esum")
                        nc.scalar.activation(e_t[:], s_t[:], AFT.Exp,
                                             accum_out=esum[:])
                        erec = sm_pool.tile([P, 1], F32, tag="erec")
                        nc.vector.reciprocal(erec[:], esum[:])
                        en_t = sm_pool.tile([P, P], F16, tag="en")
                        nc.vector.tensor_scalar_mul(en_t[:], e_t[:], erec[:])
                        # QT_m = EN_m.T @ PT_m
                        qt_ps = ps_smalls[b].tile([P, C], F32, tag="small")
                        nc.tensor.matmul(qt_ps[:], en_t[:], pt_t[:, m, :],
                                         start=True, stop=True)
                        nc.vector.tensor_copy(qt_t[:, m, :], qt_ps[:])

                    # v bias: bveff[:, oc] = Wv.T @ shift + bv
                    bveff2 = pb_pool.tile([P, NCH, 2], F16, tag="bveff2")
                    for oc in range(NCH):
                        bv_ps = ps_smalls[b].tile([P, 2], F32, tag="small")
                        for m in range(NCH):
                            nc.tensor.matmul(bv_ps[:], wv_t[:, m, oc * P:(oc + 1) * P],
                                             s["shiftw"][:, m, :],
                                             start=(m == 0), stop=(m == NCH - 1))
                        bveff = pb_pool.tile([P, 1], F32, tag="bveff")
                        nc.vector.tensor_tensor(bveff[:], bv_ps[:, 0:1],
                                                bv_t[:, oc:oc + 1], AOT.add)
                        nc.vector.tensor_copy(bveff2[:, oc, 0:1], bveff[:])
                        nc.vector.tensor_copy(bveff2[:, oc, 1:2], bveff[:])

                    # M2T + fold:  m2tot = scale * M2T + I
                    m2tot = pb_pool.tile([P, NCH, C], F16, tag="m2t")
                    for cc in range(NCH):
                        m2_ps = ps_smalls[b].tile([P, C], F32, tag="small")
                        for pjc in range(NCH):
                            nc.tensor.matmul(m2_ps[:], wvr_t[:, pjc, cc * P:(cc + 1) * P],
                                             qt_t[:, pjc, :],
                                             start=(pjc == 0), stop=(pjc == NCH - 1))
                        nc.vector.scalar_tensor_tensor(
                            m2tot[:, cc, :], m2_ps[:], s["scale"][cc][:],
                            rmask_t[:, cc, :], op0=AOT.mult, op1=AOT.add)
                    s["m2tot"] = m2tot
                    # pbeff = proj_b + QT.T @ bveff
                    pbeff = pb_pool.tile([P, NCH], F32, tag="pbeff")
                    for oc in range(NCH):
                        pbe_ps = ps_smalls[b].tile([P, 2], F32, tag="small")
                        for pjc in range(NCH):
                            nc.tensor.matmul(pbe_ps[:], qt_t[:, pjc, oc * P:(oc + 1) * P],
                                             bveff2[:, pjc, :],
                                             start=(pjc == 0), stop=(pjc == NCH - 1))
                        nc.vector.tensor_tensor(pbeff[:, oc:oc + 1], pbe_ps[:, 0:1],
                                                pb_t[:, oc:oc + 1], AOT.add)
                    s["pbeff"] = pbeff

                def stageD(b):
                    """y = M2tot.T @ x + pbeff, evict, store."""
                    s = st[b]
                    m2tot, x_r, pbeff = s["m2tot"], s["xr"], s["pbeff"]
                    NYQ = N // YQ          # 1024 cols per store chunk
                    for oc in range(NCH):
                        for q in range(YQ):
                            y_sb = y_pool.tile([P, NYQ], F16, tag="y", name="y_sb")
                            for sub in range(NYQ // 512):
                                col0 = q * NYQ + sub * 512
                                y_ps = ps_y.tile([P, 512], F32, tag="y", name="y_ps")
                                for m in range(Y_M):
                                    nc.tensor.matmul(y_ps[:],
                                                     m2tot[:, m, oc * P:(oc + 1) * P],
                                                     x_r[:, m, col0:col0 + 512],
                                                     start=(m == 0), stop=(m == Y_M - 1))
                                if (q * 2 + sub + oc) % 2 == 0:
                                    nc.scalar.activation(
                                        y_sb[:, sub * 512:(sub + 1) * 512], y_ps[:],
                                        AFT.Identity, bias=pbeff[:, oc:oc + 1])
                                else:
                                    nc.vector.tensor_scalar_add(
                                        y_sb[:, sub * 512:(sub + 1) * 512], y_ps[:],
                                        pbeff[:, oc:oc + 1])
                            if NOYD:
                                continue
                            if Y_ENG == "pool":
                                y_eng = [nc.gpsimd, nc.sync, nc.scalar][(oc * YQ + q + b) % 3]
                            else:
                                y_eng = nc.sync if (oc * YQ + q) % 2 == 0 else nc.scalar
                            y_eng.dma_start(
                                y_d[b, oc * P:(oc + 1) * P, q * NYQ:(q + 1) * NYQ],
                                y_sb[:])

                if NOCHAIN:
                    for b in range(NB):
                        stageA(b)
                        y_sb = y_pool.tile([P, 64], F16, tag="ydum", name=f"yd_{b}")
                        nc.vector.memset(y_sb[:], 0.0)
                        nc.gpsimd.dma_start(y_d[b, 0:P, 0:64], y_sb[:])
                elif NOBC:
                    for b in range(NB):
                        stageA(b)
                        s = st[b]
                        m2tot = pb_pool.tile([P, NCH, C], F16, tag="m2t", name=f"m2d_{b}")
                        nc.vector.memset(m2tot[:], 0.01)
                        pbeff = pb_pool.tile([P, NCH], F32, tag="pbeff", name=f"pbd_{b}")
                        nc.vector.memset(pbeff[:], 0.0)
                        s["m2tot"], s["pbeff"] = m2tot, pbeff
                    for b in range(NB):
                        stageD(b)
                else:
                    for b in range(NB):
                        stageA(b)
                        stageB(b)
                        stageC(b)
                    for b in range(NB):
                        stageD(b)

    if not nc.is_finalized():
        nc.finalize()
    return nc


_NC_CACHE = {}


def _get_nc(replicate=1, loop=1, prec=None):
    key = (replicate, loop, XR_ENG, Y_ENG, FAKE_XT, TQ, NOSTATS, GRAM_T, Y_M, NOYD, TMODE, NOCHAIN, TSPLIT, NOBC)
    if key not in _NC_CACHE:
        _NC_CACHE[key] = _build(replicate, loop)
    return _NC_CACHE[key]


def _host_inputs(x, norm_w, norm_b, qkv_w, qkv_b, proj_w, proj_b):
    """Host-side weight preprocessing -> per-core common input dict."""
    f = np.float32
    norm_w, norm_b = np.asarray(norm_w, f), np.asarray(norm_b, f)
    qkv_w, qkv_b = np.asarray(qkv_w, f), np.asarray(qkv_b, f)
    proj_w, proj_b = np.asarray(proj_w, f), np.asarray(proj_b, f)

    perm = ORIG_OF_PM
    wq = qkv_w[0:C][perm] / 8.0          # fold attention scale dh^-0.5 = 1/8
    wk = qkv_w[C:2 * C][perm]
    wv = qkv_w[2 * C:3 * C][perm]
    bq = qkv_b[0:C][perm] / 8.0
    bk = qkv_b[C:2 * C][perm]
    bv = qkv_b[2 * C:3 * C][perm]

    wqk = np.concatenate([wq.T, wk.T], axis=1).astype(f)      # [C, 512]
    bqk = np.concatenate([bq, bk])[None, :].astype(f)         # [1, 512]
    wv_c = np.ascontiguousarray(wv.T).astype(f)               # [C, C] (c_in, o_pm)
    pt = np.ascontiguousarray(proj_w[:, perm].T).astype(np.float16)

    ch = np.arange(C)
    ind = np.zeros((P, NCH, G), f)
    bc = np.zeros((G, NCH, P), f)
    for m in range(NCH):
        grp = (ch[m * P:(m + 1) * P]) // (C // G)
        for c0 in range(P):
            ind[c0, m, grp[c0]] = 1.0 / (C // G)
            bc[grp[c0], m, c0] = 1.0
    a = np.arange(P)
    mask = np.where((a[:, None] // NH) == (a[None, :] // NH), 0.0, MASK_NEG).astype(f)
    rmask = np.zeros((P, NCH, C), f)
    for cc in range(NCH):
        rmask[a, cc, cc * P + a] = 1.0

    def chunk2(v_):  # [C] -> [P, NCH]
        return np.stack([v_[m * P:(m + 1) * P] for m in range(NCH)], axis=1).astype(f)

    return {
        "wqk": wqk, "wv": wv_c,
        "wvr": np.ascontiguousarray(wv).astype(np.float16),
        "pt": pt, "bqk": bqk,
        "bv": chunk2(bv), "pb": chunk2(proj_b),
        "nw": chunk2(norm_w), "nb": chunk2(norm_b),
        "ind": ind, "bc": bc, "mask": mask, "rmask": rmask,
        "ident": np.eye(P, dtype=np.float16),
    }


def make_in_maps(x, norm_w, norm_b, qkv_w, qkv_b, proj_w, proj_b, prec=None):
    common = _host_inputs(x, norm_w, norm_b, qkv_w, qkv_b, proj_w, proj_b)
    xr = np.ascontiguousarray(
        np.asarray(x, dtype=np.float32).reshape(B, C, N).astype(np.float16))
    in_maps = []
    for c in range(NCORES):
        m = dict(common)
        m["x"] = xr[c * NB:(c + 1) * NB]
        in_maps.append(m)
    return in_maps


def kernel(x, norm_w, norm_b, qkv_w, qkv_b, proj_w, proj_b):
    nc = _get_nc()
    in_maps = make_in_maps(x, norm_w, norm_b, qkv_w, qkv_b, proj_w, proj_b)
    res = run_bass_kernel_spmd(nc, in_maps, core_ids=list(range(NCORES)))
    y = np.concatenate([res.results[c]["y"] for c in range(NCORES)], axis=0)
    return y.reshape(B, C, H, W).astype(np.float32)


# revision 21
# speedup vs baseline: 6.3756x; 6.3756x over previous
"""AttentionBlock (GroupNorm + degenerate head-axis attention + proj + residual)
on 8 Trainium2 NeuronCores, data-parallel over batch (2 batches per core).

Reference math (B=16, C=256, H=W=64, NH=4, dh=64, N=HW=4096, G=8 groups):
  xn   = GroupNorm(8, C)(x) * norm_w + norm_b       (xn = D x + s per channel)
  qkv  = qkv_w @ xn + qkv_b                         (1x1 convs == channel GEMMs)
  q,k,v: [NH, dh, N]; attn[p,i,j] = softmax_j( (1/8) sum_n q[i,p,n] k[j,p,n] )
  out[(p,i), n] = sum_j attn[p,i,j] v[(p,j), n]
  y    = proj_w @ out + proj_b + x

v2 kernel strategy (per core, per batch) — Gram-based:
  - xT is produced by DMA-transpose (xbar) straight from DRAM: xt[p,t,c] =
    x[c, t*128+p]; the PE accumulates the raw-x channel Gram G = X X^T in
    PSUM from the 32 transposed chunks (f16 operands, f32 accumulate)
  - GroupNorm never touches the big data: the diag scale D folds into the
    tiny q/k weight GEMMs, A = (Wq D) G (Wk D)^T + exact rank-1 bias terms
    (f32r full-rate matmuls), softmax over the pm-block-diagonal as before
  - attention+value+proj collapse to M2T[c,o] (tiny GEMMs); then BOTH the
    GroupNorm scale and the residual fold into it:  M2tot = D M2T + I, so
      y = M2tot^T @ x + pbeff      (one big GEMM over raw f16 x, no x_s,
    no identity-residual matmul, all biases in the per-channel pbeff)
  - per-channel stats for D come from bn_stats on the normal-layout x copy,
    which the y-GEMM needs anyway as its streaming operand
"""
import numpy as np

import concourse.bacc as bacc
import concourse.mybir as mybir
import concourse.tile as tile
from concourse.bass_utils import run_bass_kernel_spmd

F32 = mybir.dt.float32
F32R = mybir.dt.float32r
F16 = mybir.dt.float16

NCORES = 8
B, C, H, W = 16, 256, 64, 64
N = H * W                    # 4096
NB = B // NCORES             # batches per core = 2
NH, DH, G = 4, 64, 8
EPS = 1e-5
P = 128
NCH = C // P                 # channel chunks = 2
NT128 = N // 128             # 32
MASK_NEG = -1.0e30

PRECISION = "f16"            # output/io dtype tag (kept for test.py printout)

# p-major channel permutation: pm index j = p*4 + h  <->  orig channel h*64 + p
_PM = np.arange(C)
ORIG_OF_PM = (_PM % NH) * DH + _PM // NH   # orig channel for p-major index

TQ = 4                       # transpose DMA chunks per batch
YQ = 4                       # y store chunks per (batch, oc)
XR_ENG = "pool"              # "pool" (SWDGE) | "hwdge"
Y_ENG = "pool"               # "pool" | "hwdge"
FAKE_XT = False              # timing-only: skip transposes, memset xt once
NOSTATS = False              # timing-only: memset stat2 instead of bn_stats
GRAM_T = NT128               # timing-only: number of gram accumulation steps
Y_M = NCH                    # timing-only: contraction chunks in y GEMM
NOYD = False                 # timing-only: skip y stores
TMODE = "pe"                 # "pe" | "dram" | "sbuf" transpose method
NOCHAIN = False              # timing-only: DMAs + gram only, dummy y store
TSPLIT = True                # issue DMA transposes on both HWDGE rings
NOBC = False                 # timing-only: skip stages B/C, dummy m2tot/pbeff


def _build(replicate=1, loop=1, prec=None):
    """loop>1 wraps the computation in a hardware For_i repeating it
    (identical result every iteration) — used only for wall-clock timing."""
    nc = bacc.Bacc()
    x_d = nc.declare_dram_parameter("x", [NB, C, N], F16, isOutput=False)
    wqk_d = nc.declare_dram_parameter("wqk", [C, 512], F32R, isOutput=False)
    wv_d = nc.declare_dram_parameter("wv", [C, C], F32R, isOutput=False)
    wvr_d = nc.declare_dram_parameter("wvr", [C, C], F16, isOutput=False)
    pt_d = nc.declare_dram_parameter("pt", [C, C], F16, isOutput=False)
    bqk_d = nc.declare_dram_parameter("bqk", [1, 512], F32, isOutput=False)
    bv_d = nc.declare_dram_parameter("bv", [P, NCH], F32, isOutput=False)
    pb_d = nc.declare_dram_parameter("pb", [P, NCH], F32, isOutput=False)
    nw_d = nc.declare_dram_parameter("nw", [P, NCH], F32, isOutput=False)
    nb_d = nc.declare_dram_parameter("nb", [P, NCH], F32, isOutput=False)
    ind_d = nc.declare_dram_parameter("ind", [P, NCH, G], F32, isOutput=False)
    bc_d = nc.declare_dram_parameter("bc", [G, NCH, P], F32, isOutput=False)
    mask_d = nc.declare_dram_parameter("mask", [P, P], F32, isOutput=False)
    rmask_d = nc.declare_dram_parameter("rmask", [P, NCH, C], F32, isOutput=False)
    ident_d = nc.declare_dram_parameter("ident", [P, P], F16, isOutput=False)
    y_d = nc.declare_dram_parameter("y", [NB, C, N], F16, isOutput=True)

    AOT = mybir.AluOpType
    AFT = mybir.ActivationFunctionType

    def f32view(ap):
        return ap.bitcast(F32) if ap.dtype == F32R else ap

    with tile.TileContext(nc) as tc:
        with (
            tc.tile_pool(name="wpool", bufs=1) as wpool,
            tc.tile_pool(name="xt", bufs=2) as xt_pool,       # transposed x
            tc.tile_pool(name="xr", bufs=2) as xr_pool,       # normal x
            tc.tile_pool(name="per_b", bufs=2) as pb_pool,    # per-batch smalls
            tc.tile_pool(name="sm", bufs=3) as sm_pool,       # softmax temps
            tc.tile_pool(name="ypool", bufs=4) as y_pool,
            tc.tile_pool(name="psg", bufs=2, space="PSUM") as ps_gram,   # G / A
            tc.tile_pool(name="psx", bufs=2, space="PSUM") as ps_xp,     # PE transpose
            tc.tile_pool(name="psy", bufs=2, space="PSUM") as ps_y,
            tc.tile_pool(name="pss0", bufs=1, space="PSUM") as ps_small0,
            tc.tile_pool(name="pss1", bufs=1, space="PSUM") as ps_small1,
        ):
            # ---- load constants ----
            wqk_t = wpool.tile([P, NCH, 512], F32R)
            nc.sync.dma_start(wqk_t[:], wqk_d.rearrange("(m p) o -> p m o", p=P))
            wv_t = wpool.tile([P, NCH, C], F32R)
            nc.sync.dma_start(wv_t[:], wv_d.rearrange("(m p) o -> p m o", p=P))
            wvr_t = wpool.tile([P, NCH, C], F16)
            nc.sync.dma_start(wvr_t[:], wvr_d.rearrange("(m p) o -> p m o", p=P))
            pt_t = wpool.tile([P, NCH, C], F16)
            nc.sync.dma_start(pt_t[:], pt_d.rearrange("(m p) o -> p m o", p=P))
            bqk_t = wpool.tile([1, 512], F32)
            nc.sync.dma_start(bqk_t[:], bqk_d[:])
            bv_t = wpool.tile([P, NCH], F32)
            nc.sync.dma_start(bv_t[:], bv_d[:])
            pb_t = wpool.tile([P, NCH], F32)
            nc.sync.dma_start(pb_t[:], pb_d[:])
            nw_t = wpool.tile([P, NCH], F32)
            nc.sync.dma_start(nw_t[:], nw_d[:])
            nb_t = wpool.tile([P, NCH], F32)
            nc.sync.dma_start(nb_t[:], nb_d[:])
            ind_t = wpool.tile([P, NCH, G], F32)
            nc.sync.dma_start(ind_t[:], ind_d[:])
            bc_t = wpool.tile([G, NCH, P], F32)
            nc.sync.dma_start(bc_t[:], bc_d[:])
            mask_t = wpool.tile([P, P], F32)
            nc.sync.dma_start(mask_t[:], mask_d[:])
            rmask_t = wpool.tile([P, NCH, C], F32)
            nc.sync.dma_start(rmask_t[:], rmask_d[:])
            ident_t = wpool.tile([P, P], F16)
            nc.sync.dma_start(ident_t[:], ident_d[:])
            eps_t = wpool.tile([G, 1], F32)
            nc.vector.memset(eps_t[:], EPS)

            import contextlib
            loop_ctx = tc.For_i(0, loop, 1) if loop > 1 else contextlib.nullcontext()
            with loop_ctx:
              for _rep in range(replicate):
                st = [dict() for _ in range(NB)]

                # ------------- input DMAs, both batches, interleaved -------------
                for b in range(NB):
                    s = st[b]
                    s["xt"] = xt_pool.tile([P, NT128, C], F16, tag="xt", name=f"xt_{b}")
                    s["xr"] = xr_pool.tile([P, NCH, N], F16, tag="xr", name=f"xr_{b}")
                xr_eng = {"pool": nc.gpsimd, "hwdge": nc.scalar}[XR_ENG]
                NTQ = NT128 // TQ
                NQ = N // TQ
                for b in range(NB):
                    s = st[b]
                    if FAKE_XT:
                        nc.vector.memset(s["xt"][:, 0:1, :], 0.001)
                    if TMODE == "pe":
                        engs = [nc.gpsimd, nc.sync, nc.scalar]
                        for i, (m, h) in enumerate([(0, 0), (0, 1), (1, 0), (1, 1)]):
                            engs[(i + 2 * b) % 3].dma_start(
                                s["xr"][:, m, h * 2048:(h + 1) * 2048],
                                x_d[b, m * P:(m + 1) * P, h * 2048:(h + 1) * 2048])
                    elif TMODE == "sbuf":
                        for m in range(NCH):
                            for h in range(2):
                                xr_eng.dma_start(
                                    s["xr"][:, m, h * 2048:(h + 1) * 2048],
                                    x_d[b, m * P:(m + 1) * P, h * 2048:(h + 1) * 2048])
                                if not FAKE_XT:
                                    nc.sync.dma_start(
                                        s["xt"][:, h * 16:(h + 1) * 16, m * P:(m + 1) * P],
                                        s["xr"][:, m, h * 2048:(h + 1) * 2048],
                                        transpose=True)
                    else:
                        for q in range(TQ // 2):
                            if not FAKE_XT:
                                teng = nc.scalar if (TSPLIT and q % 2 == 1) else nc.sync
                                teng.dma_start(
                                    s["xt"][:, q * NTQ:(q + 1) * NTQ, :],
                                    x_d[b, :, q * NQ:(q + 1) * NQ], transpose=True)
                        xr_eng.dma_start(s["xr"][:, 0, :], x_d[b, 0:P, :])
                        for q in range(TQ // 2, TQ):
                            if not FAKE_XT:
                                teng = nc.scalar if (TSPLIT and q % 2 == 1) else nc.sync
                                teng.dma_start(
                                    s["xt"][:, q * NTQ:(q + 1) * NTQ, :],
                                    x_d[b, :, q * NQ:(q + 1) * NQ], transpose=True)
                        xr_eng.dma_start(s["xr"][:, 1, :], x_d[b, P:C, :])

                ps_smalls = [ps_small0, ps_small1]

                # ------------- per-batch pipeline -------------
                def stageA(b):
                    """Gram accumulation (PE) + channel stats (DVE)."""
                    s = st[b]
                    xt, x_r = s["xt"], s["xr"]
                    # one accumulation chain per PSUM bank (full-bank tiles):
                    # a start=True matmul clears has_written for the whole 2KB
                    # zero region, so chains must never share a bank.
                    g_ps = [ps_gram.tile([P, 512], F32, tag="ga", name=f"g_{b}_{m}")
                            for m in range(NCH)]
                    if TMODE == "pe" and not FAKE_XT:
                        # two t-chunks per PSUM bank (4 single-MM chains), one
                        # [P, 512] f16 copy per pair
                        for tp in range(NT128 // 2):
                            xp = ps_xp.tile([P, 512], F32, tag="xp",
                                            name=f"xp_{b}_{tp}")
                            for half in range(2):
                                t = 2 * tp + half
                                for m in range(NCH):
                                    nc.tensor.matmul(
                                        xp[:, half * 256 + m * P:
                                           half * 256 + (m + 1) * P],
                                        x_r[:, m, t * P:(t + 1) * P],
                                        ident_t[:], start=True, stop=True)
                            if tp % 2 == 0:
                                nc.vector.tensor_copy(xt[:, 2 * tp:2 * tp + 2, :],
                                                      xp[:])
                            else:
                                nc.scalar.copy(xt[:, 2 * tp:2 * tp + 2, :], xp[:])
                    for t in range(GRAM_T):
                        tt = 0 if FAKE_XT else t % NT128
                        for m in range(NCH):
                            nc.tensor.matmul(g_ps[m][:, 0:256],
                                             xt[:, tt, m * P:(m + 1) * P],
                                             xt[:, tt, :],
                                             start=(t == 0), stop=(t == GRAM_T - 1))
                    s["g_ps"] = g_ps
                    # per-channel stats: stat2 = (mean, E[x^2])
                    stat2 = pb_pool.tile([P, NCH, 2], F32, tag="stat2")
                    if NOSTATS:
                        nc.vector.memset(stat2[:], 1.0)
                    else:
                        for m in range(NCH):
                            stats = pb_pool.tile([P, 8, 6], F32, tag="stats")
                            for j in range(8):
                                nc.vector.bn_stats(stats[:, j, :],
                                                   x_r[:, m, j * 512:(j + 1) * 512])
                            mv = pb_pool.tile([P, 2], F32, tag="mv")
                            nc.vector.bn_aggr(mv[:], stats[:])
                            nc.vector.tensor_copy(stat2[:, m, 0:1], mv[:, 0:1])
                            sq = pb_pool.tile([P, 1], F32, tag="sq")
                            nc.vector.tensor_mul(sq[:], mv[:, 0:1], mv[:, 0:1])
                            nc.vector.tensor_add(stat2[:, m, 1:2], mv[:, 1:2], sq[:])
                    s["stat2"] = stat2

                def stageB(b):
                    """Group stats -> scale/shift; wqkD; G->SBUF; T and A (+rank-1)."""
                    s = st[b]
                    stat2 = s["stat2"]
                    shiftw = pb_pool.tile([P, NCH, 2], F32R, tag="shiftw")
                    sxs2 = pb_pool.tile([P, NCH, 2], F32R, tag="sxs2")
                    wqkD = pb_pool.tile([P, NCH, 512], F32R, tag="wqkD")
                    scale_keep = []
                    # group stats: one chain covers all 8 groups (each group's
                    # channels live in exactly one chunk; ind zeros elsewhere)
                    sg_ps = ps_smalls[b].tile([G, 2], F32, tag="small")
                    for m in range(NCH):
                        nc.tensor.matmul(sg_ps[:], ind_t[:, m, :], stat2[:, m, :],
                                         start=(m == 0), stop=(m == NCH - 1))
                    gs = pb_pool.tile([G, 2], F32, tag="gs")
                    nc.vector.tensor_copy(gs[:], sg_ps[:])
                    gsq = pb_pool.tile([G, 1], F32, tag="gsq")
                    nc.vector.tensor_mul(gsq[:], gs[:, 0:1], gs[:, 0:1])
                    gvar = pb_pool.tile([G, 1], F32, tag="gvar")
                    nc.vector.tensor_tensor(gvar[:], gs[:, 1:2], gsq[:], AOT.subtract)
                    grt = pb_pool.tile([G, 1], F32, tag="grt")
                    nc.scalar.activation(grt[:], gvar[:], AFT.Ln, bias=eps_t[:])
                    grtm = pb_pool.tile([G, 1], F32, tag="grtm")
                    nc.vector.tensor_scalar_mul(grtm[:], grt[:], -0.5)
                    gstats = pb_pool.tile([G, 2], F32, tag="gstats")
                    nc.vector.tensor_copy(gstats[:, 0:1], gs[:, 0:1])
                    nc.scalar.activation(gstats[:, 1:2], grtm[:], AFT.Exp)
                    for m in range(NCH):
                        # per-channel scale/shift for chunk m
                        bc_ps = ps_smalls[b].tile([P, 2], F32, tag="small")
                        nc.tensor.matmul(bc_ps[:], bc_t[:, m, :], gstats[:],
                                         start=True, stop=True)
                        scale_m = pb_pool.tile([P, 1], F32, tag="scale_m")
                        nc.vector.tensor_mul(scale_m[:], bc_ps[:, 1:2], nw_t[:, m:m + 1])
                        tmp_m = pb_pool.tile([P, 1], F32, tag="tmp_m")
                        nc.vector.tensor_mul(tmp_m[:], bc_ps[:, 0:1], scale_m[:])
                        # shift = norm_b - mean*scale (duplicated to width 2)
                        nc.vector.tensor_tensor(shiftw[:, m, 0:1],
                                                nb_t[:, m:m + 1], tmp_m[:], AOT.subtract)
                        nc.vector.tensor_tensor(shiftw[:, m, 1:2],
                                                nb_t[:, m:m + 1], tmp_m[:], AOT.subtract)
                        # sxs = scale * sum_n x = scale * N * mean
                        sxs_m = pb_pool.tile([P, 1], F32, tag="sxs_m")
                        nc.vector.tensor_mul(sxs_m[:], stat2[:, m, 0:1], scale_m[:])
                        nc.vector.tensor_scalar_mul(sxs2[:, m, 0:1], sxs_m[:], float(N))
                        nc.vector.tensor_scalar_mul(sxs2[:, m, 1:2], sxs_m[:], float(N))
                        # D-scaled q/k weights for this contraction chunk
                        nc.vector.tensor_scalar_mul(wqkD[:, m, :],
                                                    f32view(wqk_t[:, m, :]), scale_m[:])
                        scale_keep.append(scale_m)
                    s["shiftw"], s["sxs2"] = shiftw, sxs2
                    s["wqkD"], s["scale"] = wqkD, scale_keep

                    # rank-1 bias vectors (fp32): beff = W.T @ shift + bqk  [1, 512]
                    bq_ps = ps_smalls[b].tile([2, 512], F32, tag="small")
                    for m in range(NCH):
                        nc.tensor.matmul(bq_ps[:], shiftw[:, m, :], wqk_t[:, m, :],
                                         start=(m == 0), stop=(m == NCH - 1))
                    bq_sb = pb_pool.tile([1, 512], F32R, tag="bq_sb")
                    nc.vector.tensor_tensor(bq_sb[:], bq_ps[0:1, :], bqk_t[:], AOT.add)
                    # wsx = W.T @ (scale * sx) -> [1, 512]
                    wsx_ps = ps_smalls[b].tile([2, 512], F32, tag="small")
                    for m in range(NCH):
                        nc.tensor.matmul(wsx_ps[:], sxs2[:, m, :], wqk_t[:, m, :],
                                         start=(m == 0), stop=(m == NCH - 1))
                    wsx_sb = pb_pool.tile([1, 512], F32R, tag="wsx_sb")
                    nc.vector.tensor_copy(wsx_sb[:], wsx_ps[0:1, :])
                    # wcomb = Wk_eff sx + N * beff_k
                    nbk = pb_pool.tile([1, C], F32, tag="nbk")
                    nc.vector.tensor_scalar_mul(nbk[:], f32view(bq_sb[0:1, C:2 * C]),
                                                float(N))
                    wcomb = pb_pool.tile([1, C], F32R, tag="wcomb")
                    nc.vector.tensor_tensor(wcomb[:], f32view(wsx_sb[0:1, C:2 * C]),
                                            nbk[:], AOT.add)
                    s["bq_sb"], s["wsx_sb"], s["wcomb"] = bq_sb, wsx_sb, wcomb

                    # G -> SBUF (fp32)
                    g_sb = pb_pool.tile([P, NCH, 256], F32R, tag="g_sb")
                    for m in range(NCH):
                        nc.vector.tensor_copy(g_sb[:, m, :], s["g_ps"][m][:, 0:256])
                    # T[c2, i] = sum_c1 G[c1, c2] WqD[c1, i]   (q-half only)
                    t_ps = [ps_smalls[b].tile([P, 512], F32, tag="small", name=f"t_{b}_{o}")
                            for o in range(NCH)]
                    for o in range(NCH):
                        for m1 in range(NCH):
                            nc.tensor.matmul(t_ps[o][:, 0:256],
                                             g_sb[:, m1, o * P:(o + 1) * P],
                                             wqkD[:, m1, 0:256],
                                             start=(m1 == 0), stop=(m1 == NCH - 1))
                    t_sb = pb_pool.tile([P, NCH, 256], F32R, tag="t_sb")
                    for o in range(NCH):
                        nc.scalar.copy(t_sb[:, o, :], t_ps[o][:, 0:256])
                    # A[i, j] = sum_c2 T[c2, i] WkD[c2, j]  + rank-1 bias terms
                    a_ps = [ps_smalls[b].tile([P, 512], F32, tag="small", name=f"a_{b}_{m}")
                            for m in range(NCH)]
                    for m in range(NCH):
                        for o in range(NCH):
                            nc.tensor.matmul(a_ps[m][:, 0:256],
                                             t_sb[:, o, m * P:(m + 1) * P],
                                             wqkD[:, o, 256:512],
                                             start=(o == 0), stop=False)
                        nc.tensor.matmul(a_ps[m][:, 0:256],
                                         bq_sb[0:1, m * P:(m + 1) * P],
                                         wcomb[:], start=False, stop=False)
                        nc.tensor.matmul(a_ps[m][:, 0:256],
                                         wsx_sb[0:1, m * P:(m + 1) * P],
                                         bq_sb[0:1, C:2 * C], start=False, stop=True)
                    s["a_ps"] = a_ps

                def stageC(b):
                    """Softmax, QT, M2T (+fold scale & residual), bveff/pbeff."""
                    s = st[b]
                    a_ps = s["a_ps"]
                    qt_t = pb_pool.tile([P, NCH, C], F16, tag="qt")
                    for m in range(NCH):
                        grel = a_ps[m][:, m * P:(m + 1) * P]
                        s_t = sm_pool.tile([P, P], F32, tag="s")
                        nc.vector.tensor_tensor(s_t[:], grel, mask_t[:], AOT.add)
                        e_t = sm_pool.tile([P, P], F32, tag="e")
                        esum = sm_pool.tile([P, 1], F32, tag="esum")
                        nc.scalar.activation(e_t[:], s_t[:], AFT.Exp,
                                             accum_out=esum[:])
                        erec = sm_pool.tile([P, 1], F32, tag="erec")
                        nc.vector.reciprocal(erec[:], esum[:])
                        en_t = sm_pool.tile([P, P], F16, tag="en")
                        nc.vector.tensor_scalar_mul(en_t[:], e_t[:], erec[:])
                        # QT_m = EN_m.T @ PT_m
                        qt_ps = ps_smalls[b].tile([P, C], F32, tag="small")
                        nc.tensor.matmul(qt_ps[:], en_t[:], pt_t[:, m, :],
                                         start=True, stop=True)
                        nc.vector.tensor_copy(qt_t[:, m, :], qt_ps[:])

                    # v bias: bveff[:, oc] = Wv.T @ shift + bv
                    bveff2 = pb_pool.tile([P, NCH, 2], F16, tag="bveff2")
                    for oc in range(NCH):
                        bv_ps = ps_smalls[b].tile([P, 2], F32, tag="small")
                        for m in range(NCH):
                            nc.tensor.matmul(bv_ps[:], wv_t[:, m, oc * P:(oc + 1) * P],
                                             s["shiftw"][:, m, :],
                                             start=(m == 0), stop=(m == NCH - 1))
                        bveff = pb_pool.tile([P, 1], F32, tag="bveff")
                        nc.vector.tensor_tensor(bveff[:], bv_ps[:, 0:1],
                                                bv_t[:, oc:oc + 1], AOT.add)
                        nc.vector.tensor_copy(bveff2[:, oc, 0:1], bveff[:])
                        nc.vector.tensor_copy(bveff2[:, oc, 1:2], bveff[:])

                    # M2T + fold:  m2tot = scale * M2T + I
                    m2tot = pb_pool.tile([P, NCH, C], F16, tag="m2t")
                    for cc in range(NCH):
                        m2_ps = ps_smalls[b].tile([P, C], F32, tag="small")
                        for pjc in range(NCH):
                            nc.tensor.matmul(m2_ps[:], wvr_t[:, pjc, cc * P:(cc + 1) * P],
                                             qt_t[:, pjc, :],
                                             start=(pjc == 0), stop=(pjc == NCH - 1))
                        nc.vector.scalar_tensor_tensor(
                            m2tot[:, cc, :], m2_ps[:], s["scale"][cc][:],
                            rmask_t[:, cc, :], op0=AOT.mult, op1=AOT.add)
                    s["m2tot"] = m2tot
                    # pbeff = proj_b + QT.T @ bveff
                    pbeff = pb_pool.tile([P, NCH], F32, tag="pbeff")
                    for oc in range(NCH):
                        pbe_ps = ps_smalls[b].tile([P, 2], F32, tag="small")
                        for pjc in range(NCH):
                            nc.tensor.matmul(pbe_ps[:], qt_t[:, pjc, oc * P:(oc + 1) * P],
                                             bveff2[:, pjc, :],
                                             start=(pjc == 0), stop=(pjc == NCH - 1))
                        nc.vector.tensor_tensor(pbeff[:, oc:oc + 1], pbe_ps[:, 0:1],
                                                pb_t[:, oc:oc + 1], AOT.add)
                    s["pbeff"] = pbeff

                def stageD(b):
                    """y = M2tot.T @ x + pbeff, evict, store."""
                    s = st[b]
                    m2tot, x_r, pbeff = s["m2tot"], s["xr"], s["pbeff"]
                    NYQ = N // YQ          # 1024 cols per store chunk
                    for oc in range(NCH):
                        for q in range(YQ):
                            y_sb = y_pool.tile([P, NYQ], F16, tag="y", name="y_sb")
                            for sub in range(NYQ // 512):
                                col0 = q * NYQ + sub * 512
                                y_ps = ps_y.tile([P, 512], F32, tag="y", name="y_ps")
                                for m in range(Y_M):
                                    nc.tensor.matmul(y_ps[:],
                                                     m2tot[:, m, oc * P:(oc + 1) * P],
                                                     x_r[:, m, col0:col0 + 512],
                                                     start=(m == 0), stop=(m == Y_M - 1))
                                if (q * 2 + sub + oc) % 2 == 0:
                                    nc.scalar.activation(
                                        y_sb[:, sub * 512:(sub + 1) * 512], y_ps[:],
                                        AFT.Identity, bias=pbeff[:, oc:oc + 1])
                                else:
                                    nc.vector.tensor_scalar_add(
                                        y_sb[:, sub * 512:(sub + 1) * 512], y_ps[:],
                                        pbeff[:, oc:oc + 1])
                            if NOYD:
                                continue
                            if Y_ENG == "pool":
                                y_eng = nc.gpsimd
                            else:
                                y_eng = nc.sync if (oc * YQ + q) % 2 == 0 else nc.scalar
                            y_eng.dma_start(
                                y_d[b, oc * P:(oc + 1) * P, q * NYQ:(q + 1) * NYQ],
                                y_sb[:])

                if NOCHAIN:
                    for b in range(NB):
                        stageA(b)
                        y_sb = y_pool.tile([P, 64], F16, tag="ydum", name=f"yd_{b}")
                        nc.vector.memset(y_sb[:], 0.0)
                        nc.gpsimd.dma_start(y_d[b, 0:P, 0:64], y_sb[:])
                elif NOBC:
                    for b in range(NB):
                        stageA(b)
                        s = st[b]
                        m2tot = pb_pool.tile([P, NCH, C], F16, tag="m2t", name=f"m2d_{b}")
                        nc.vector.memset(m2tot[:], 0.01)
                        pbeff = pb_pool.tile([P, NCH], F32, tag="pbeff", name=f"pbd_{b}")
                        nc.vector.memset(pbeff[:], 0.0)
                        s["m2tot"], s["pbeff"] = m2tot, pbeff
                    for b in range(NB):
                        stageD(b)
                else:
                    for b in range(NB):
                        stageA(b)
                        stageB(b)
                        stageC(b)
                    for b in range(NB):
                        stageD(b)

    if not nc.is_finalized():
        nc.finalize()
    return nc


_NC_CACHE = {}


def _get_nc(replicate=1, loop=1, prec=None):
    key = (replicate, loop, XR_ENG, Y_ENG, FAKE_XT, TQ, NOSTATS, GRAM_T, Y_M, NOYD, TMODE, NOCHAIN, TSPLIT, NOBC)
    if key not in _NC_CACHE:
        _NC_CACHE[key] = _build(replicate, loop)
    return _NC_CACHE[key]


def _host_inputs(x, norm_w, norm_b, qkv_w, qkv_b, proj_w, proj_b):
    """Host-side weight preprocessing -> per-core common input dict."""
    f = np.float32
    norm_w, norm_b = np.asarray(norm_w, f), np.asarray(norm_b, f)
    qkv_w, qkv_b = np.asarray(qkv_w, f), np.asarray(qkv_b, f)
    proj_w, proj_b = np.asarray(proj_w, f), np.asarray(proj_b, f)

    perm = ORIG_OF_PM
    wq = qkv_w[0:C][perm] / 8.0          # fold attention scale dh^-0.5 = 1/8
    wk = qkv_w[C:2 * C][perm]
    wv = qkv_w[2 * C:3 * C][perm]
    bq = qkv_b[0:C][perm] / 8.0
    bk = qkv_b[C:2 * C][perm]
    bv = qkv_b[2 * C:3 * C][perm]

    wqk = np.concatenate([wq.T, wk.T], axis=1).astype(f)      # [C, 512]
    bqk = np.concatenate([bq, bk])[None, :].astype(f)         # [1, 512]
    wv_c = np.ascontiguousarray(wv.T).astype(f)               # [C, C] (c_in, o_pm)
    pt = np.ascontiguousarray(proj_w[:, perm].T).astype(np.float16)

    ch = np.arange(C)
    ind = np.zeros((P, NCH, G), f)
    bc = np.zeros((G, NCH, P), f)
    for m in range(NCH):
        grp = (ch[m * P:(m + 1) * P]) // (C // G)
        for c0 in range(P):
            ind[c0, m, grp[c0]] = 1.0 / (C // G)
            bc[grp[c0], m, c0] = 1.0
    a = np.arange(P)
    mask = np.where((a[:, None] // NH) == (a[None, :] // NH), 0.0, MASK_NEG).astype(f)
    rmask = np.zeros((P, NCH, C), f)
    for cc in range(NCH):
        rmask[a, cc, cc * P + a] = 1.0

    def chunk2(v_):  # [C] -> [P, NCH]
        return np.stack([v_[m * P:(m + 1) * P] for m in range(NCH)], axis=1).astype(f)

    return {
        "wqk": wqk, "wv": wv_c,
        "wvr": np.ascontiguousarray(wv).astype(np.float16),
        "pt": pt, "bqk": bqk,
        "bv": chunk2(bv), "pb": chunk2(proj_b),
        "nw": chunk2(norm_w), "nb": chunk2(norm_b),
        "ind": ind, "bc": bc, "mask": mask, "rmask": rmask,
        "ident": np.eye(P, dtype=np.float16),
    }


def make_in_maps(x, norm_w, norm_b, qkv_w, qkv_b, proj_w, proj_b, prec=None):
    common = _host_inputs(x, norm_w, norm_b, qkv_w, qkv_b, proj_w, proj_b)
    xr = np.ascontiguousarray(
        np.asarray(x, dtype=np.float32).reshape(B, C, N).astype(np.float16))
    in_maps = []
    for c in range(NCORES):
        m = dict(common)
        m["x"] = xr[c * NB:(c + 1) * NB]
        in_maps.append(m)
    return in_maps


def kernel(x, norm_w, norm_b, qkv_w, qkv_b, proj_w, proj_b):
    nc = _get_nc()
    in_maps = make_in_maps(x, norm_w, norm_b, qkv_w, qkv_b, proj_w, proj_b)
    res = run_bass_kernel_spmd(nc, in_maps, core_ids=list(range(NCORES)))
    y = np.concatenate([res.results[c]["y"] for c in range(NCORES)], axis=0)
    return y.reshape(B, C, H, W).astype(np.float32)
